# revision 29
# baseline (speedup 1.0000x reference)
"""Trainium2 Bass kernel for the bipartite GNN message-passing encoder.

Math (see reference.py):
  A_r = (adj == r), r = 1..5
  An_r = diag(a) A_r diag(b),  a_u = rsqrt(max(Nu,1)), b_i = rsqrt(max(Nv,1))
  Hu = relu(sum_r An_r @ W_items_r^T)   [NU, M]
  Hv = relu(sum_r An_r^T @ W_users_r^T) [NI, M]
  U  = relu(Hu @ dense_W^T + relu(u_sideFeat @ u_W1^T + u_b1) @ u_W2^T)
  V  = relu(Hv @ dense_W^T + relu(v_sideFeat @ v_W1^T + v_b1) @ v_W2^T)

Sharding ("collective-free" 8-way): core c owns users [500c, 500c+500) and
items [500c, 500c+500), padded per-core to 512 (padded global index
512c + j, NP = 4096 total).  Each core holds the FULL contraction data for
its rows: adj[:, I_c] u-major (for Hv) and adj[U_c, :]^T i-major (for Hu),
plus the full msg_W both ways (host-packed bf16).  It computes
HvT[m, own-items] / HuT[m, own-users] completely locally, so there are NO
big AllReduces: only a 16KB degree AllGather (measured faster than an
AllReduce as the first collective: 14-17us vs 17-22us; the 8-rank sum is
done on-device with one rearranged-AP DMA and a DVE/gpsimd tree) plus one
hidden 16KB AllReduce for the item side.  Degrees ride fused zero-count passes
split across DVE (is_equal 0) and ACT (Relu(1 - adj)), both with a
free-axis accum_out (deg = 4000 - sum of zero counts over the 8 cores;
pad rows read 0 and drop out of every product).  Masks are built on DVE
with the contraction-side degree factor folded in via the dual-op; the
output-side factor is applied in pass 2 per-partition (own-core factors
extracted from the AllReduce result with a data-driven one-hot select,
keeping the program SPMD-uniform).  The matmul stream is 640 back-to-back
500-col matmuls into 2 PSUM banks per chain (measured ~2.0 cols/ns
sustained under the HAM duty throttle; 250-col/4-bank and fp8-adjacency
variants both measured slower).  pass2 for the v side is emitted between
the chains so its latency hides under the Hu stream; the side-feature term
is pre-scaled by sqrt(deg_own) and injected into the pass-2 PSUM with an
identity-lhsT matmul, so one fused scale+relu ACT op finishes each row
(relu(s*pa + F) == relu(s*(pa + F*sqrt(deg)))).

Host-side prep (allowed layout work only: slice/pad/transpose/dtype):
everything arrives bf16, pre-transposed, chunk-packed ([128, 32*S] with
element (p, c*S+j) = src[c*128+p, j]) so every big DMA is one contiguous
descriptor.
"""

import sys

import numpy as np

if "/opt/trn_rl_repo" not in sys.path:
    sys.path.insert(0, "/opt/trn_rl_repo")

import concourse.bacc as bacc  # noqa: E402
import concourse.mybir as mybir  # noqa: E402
import concourse.tile as tile  # noqa: E402
from concourse.masks import make_identity  # noqa: E402

FP = mybir.dt.float32
BF = mybir.dt.bfloat16

NU = NI = 4000
R = 5
M = 256
OUT = 75
SIDE = 64
FDIM = 128

NCORES = 8
SO = 500        # owned users/items per core
SP = 512        # padded owned span
NP = 4096       # padded global span
CH = NP // 128  # 32 chunks of 128 along the contraction dim

AF = mybir.ActivationFunctionType
ALU = mybir.AluOpType
WORLD = [list(range(NCORES))]


def build_program():
    from contextlib import ExitStack

    nc = bacc.Bacc("TRN2", target_bir_lowering=False, debug=False, num_devices=NCORES)

    # ---- I/O ----
    # adj_u: chunk-packed [128, CH*SP] from adj_pad[:, I_c]   (u on partitions)
    # adj_i: chunk-packed [128, CH*SP] from adj_pad[U_c, :]^T (i on partitions)
    adj_u = nc.dram_tensor("adj_u", [128, CH * SP], BF, kind="ExternalInput")
    adj_i = nc.dram_tensor("adj_i", [128, CH * SP], BF, kind="ExternalInput")
    # wu/wi: chunk-packed [R, 128, CH*M] from msg_W slices, pre-transposed
    wu = nc.dram_tensor("wu", [R, 128, CH * M], BF, kind="ExternalInput")
    wi = nc.dram_tensor("wi", [R, 128, CH * M], BF, kind="ExternalInput")
    sfu = nc.dram_tensor("sfu", [FDIM, SP], BF, kind="ExternalInput")
    sfv = nc.dram_tensor("sfv", [FDIM, SP], BF, kind="ExternalInput")
    dwt = nc.dram_tensor("dwt", [M, OUT], BF, kind="ExternalInput")
    uw1t = nc.dram_tensor("uw1t", [FDIM, SIDE], BF, kind="ExternalInput")
    ub1 = nc.dram_tensor("ub1", [SIDE, 1], FP, kind="ExternalInput")
    uw2t = nc.dram_tensor("uw2t", [SIDE, OUT], BF, kind="ExternalInput")
    vw1t = nc.dram_tensor("vw1t", [FDIM, SIDE], BF, kind="ExternalInput")
    vb1 = nc.dram_tensor("vb1", [SIDE, 1], FP, kind="ExternalInput")
    vw2t = nc.dram_tensor("vw2t", [SIDE, OUT], BF, kind="ExternalInput")
    # selb: [128, 4*CH] one-hot select blocks; block j column (4c+j) is 1.0
    selb = nc.dram_tensor("selb", [128, 4 * CH], FP, kind="ExternalInput")
    u_out = nc.dram_tensor("u_out", [SO, OUT], FP, kind="ExternalOutput")
    v_out = nc.dram_tensor("v_out", [SO, OUT], FP, kind="ExternalOutput")

    with tile.TileContext(nc) as tc, ExitStack() as ctx:
        res = ctx.enter_context(tc.tile_pool(name="res", bufs=1))
        scr = ctx.enter_context(tc.tile_pool(name="scr", bufs=2))
        wpool = ctx.enter_context(tc.tile_pool(name="wpool", bufs=6))
        dram = ctx.enter_context(tc.tile_pool(name="dram", bufs=1, space="DRAM"))
        ps_chain = ctx.enter_context(tc.tile_pool(name="ps_chain", bufs=4, space="PSUM"))
        ps_small = ctx.enter_context(tc.tile_pool(name="ps_small", bufs=4, space="PSUM"))

        # ---------- bulk DMA issue (tensor-engine queue; PE idle pre-stream) ----
        # transfer order = priority: adj (degree gate) > smalls > weights
        adj_u_sb = []
        adj_i_sb = []
        for k in range(4):
            t = res.tile([128, 8 * SP], BF, tag=f"adju{k}", name="adju")
            nc.gpsimd.dma_start(out=t[:, :], in_=adj_u[:, k * 8 * SP : (k + 1) * 8 * SP])
            adj_u_sb.append(t)
        for k in range(4):
            t = res.tile([128, 8 * SP], BF, tag=f"adji{k}", name="adji")
            nc.gpsimd.dma_start(out=t[:, :], in_=adj_i[:, k * 8 * SP : (k + 1) * 8 * SP])
            adj_i_sb.append(t)

        sfu_sb = res.tile([FDIM, SP], BF, tag="sfu")
        nc.gpsimd.dma_start(out=sfu_sb[:, :], in_=sfu[:, :])
        sfv_sb = res.tile([FDIM, SP], BF, tag="sfv")
        nc.gpsimd.dma_start(out=sfv_sb[:, :], in_=sfv[:, :])
        dwt_sb = []
        for mh in range(2):
            t = res.tile([128, OUT], BF, tag=f"dwt{mh}")
            nc.gpsimd.dma_start(out=t[:, :], in_=dwt[mh * 128 : (mh + 1) * 128, :])
            dwt_sb.append(t)
        uw1t_sb = res.tile([FDIM, SIDE], BF, tag="uw1t")
        nc.gpsimd.dma_start(out=uw1t_sb[:, :], in_=uw1t[:, :])
        uw2t_sb = res.tile([SIDE, OUT], BF, tag="uw2t")
        nc.gpsimd.dma_start(out=uw2t_sb[:, :], in_=uw2t[:, :])
        vw1t_sb = res.tile([FDIM, SIDE], BF, tag="vw1t")
        nc.gpsimd.dma_start(out=vw1t_sb[:, :], in_=vw1t[:, :])
        vw2t_sb = res.tile([SIDE, OUT], BF, tag="vw2t")
        nc.gpsimd.dma_start(out=vw2t_sb[:, :], in_=vw2t[:, :])
        ub1_sb = res.tile([SIDE, 1], FP, tag="ub1")
        nc.gpsimd.dma_start(out=ub1_sb[:, :], in_=ub1[:, :])
        vb1_sb = res.tile([SIDE, 1], FP, tag="vb1")
        nc.gpsimd.dma_start(out=vb1_sb[:, :], in_=vb1[:, :])
        selb_sb = res.tile([128, 4 * CH], FP, tag="selb")
        nc.gpsimd.dma_start(out=selb_sb[:, :], in_=selb[:, :])
        ident_sb = res.tile([128, 128], BF, tag="ident")
        make_identity(nc, ident_sb[:, :])

        wtiles = []
        for r in range(R):
            t = wpool.tile([128, CH * M], BF, tag="w", name="wt")
            nc.gpsimd.dma_start(out=t[:, :], in_=wu[r, :, :])
            wtiles.append(t)
        witiles = []
        t = wpool.tile([128, CH * M], BF, tag="w", name="wt")
        nc.gpsimd.dma_start(out=t[:, :], in_=wi[0, :, :])
        witiles.append(t)

        # ---------- degree zero-count pass + tiny world AllReduces ------------
        # Split across DVE (is_equal 0) and ACT (Relu(1-x)); both with a
        # free-axis accum_out giving per-row zero counts over the own span.
        # The LAST-started core's zcu path gates the AllReduce (launch skew),
        # so this latency is on the critical path.
        def zc_pass(adj_sb, zc, start=0, upto=CH):
            for c in range(start, upto):
                sl = adj_sb[c // 8][:, (c % 8) * SP : (c % 8) * SP + SO]
                if c % 8 < 5:
                    tscr = scr.tile([128, SO], FP, tag="tscrv", bufs=3, name="tscr")
                    nc.vector.tensor_scalar(
                        out=tscr[:, :], in0=sl, scalar1=0.0, scalar2=None,
                        op0=ALU.is_equal, op1=ALU.add, accum_out=zc[:, c : c + 1],
                    )
                else:
                    tscr = scr.tile([128, SO], FP, tag="tscrs", bufs=3, name="tscr")
                    nc.scalar.activation(
                        out=tscr[:, :], in_=sl,
                        func=AF.Relu, scale=-1.0, bias=1.0,
                        accum_out=zc[:, c : c + 1],
                    )

        zcu = res.tile([128, CH], FP, tag="zcu")
        zci = res.tile([128, CH], FP, tag="zci")
        zc_pass(adj_u_sb, zcu)
        # contribute (SO - zc): the gathered sum is then the degree directly
        zcun = res.tile([128, CH], FP, tag="zcun")
        nc.vector.tensor_scalar(
            out=zcun[:, :], in0=zcu[:, :], scalar1=-1.0, scalar2=float(SO),
            op0=ALU.mult, op1=ALU.add,
        )
        dram_zcu = dram.tile([128, CH], FP, tag="dram_zcu")
        dram_zcu_ag = dram.tile([NCORES * 128, CH], FP, tag="dram_zcu_ag")
        nc.sync.dma_start(out=dram_zcu[:, :], in_=zcun[:, :])
        nc.gpsimd.collective_compute(
            "AllGather", ALU.bypass, replica_groups=WORLD,
            ins=[dram_zcu.opt()], outs=[dram_zcu_ag.opt()],
        )
        zc_pass(adj_i_sb, zci)
        zcin = res.tile([128, CH], FP, tag="zcin")
        nc.vector.tensor_scalar(
            out=zcin[:, :], in0=zci[:, :], scalar1=-1.0, scalar2=float(SO),
            op0=ALU.mult, op1=ALU.add,
        )
        dram_zci = dram.tile([128, CH], FP, tag="dram_zci")
        dram_zci_red = dram.tile([128, CH], FP, tag="dram_zci_red")
        nc.sync.dma_start(out=dram_zci[:, :], in_=zcin[:, :])
        nc.gpsimd.collective_compute(
            "AllReduce", ALU.add, replica_groups=WORLD,
            ins=[dram_zci.opt()], outs=[dram_zci_red.opt()],
        )

        # ---------- side-feature pass-2 prep (independent of collectives) -----
        def side_prep(w1t_sb, b1_sb, sf_sb, w2t_sb, nm):
            pf = ps_small.tile([128, SP], FP, tag="sm", name="pf")
            nc.tensor.matmul(
                pf[:SIDE, :SO], lhsT=w1t_sb[:, :], rhs=sf_sb[:, :SO],
                start=True, stop=True,
            )
            fT = res.tile([SIDE, SO], BF, tag=f"fT{nm}", name="fT")
            nc.scalar.activation(
                out=fT[:, :], in_=pf[:SIDE, :SO], func=AF.Relu, bias=b1_sb[:, :]
            )
            fs = []
            for ic in range(4):
                w = min(128, SO - ic * 128)
                pfs = ps_small.tile([128, SP], FP, tag="sm", name="pfs")
                nc.tensor.matmul(
                    pfs[:w, :OUT], lhsT=fT[:, ic * 128 : ic * 128 + w],
                    rhs=w2t_sb[:, :], start=True, stop=True,
                )
                t = res.tile([128, OUT], FP, tag=f"fs{nm}{ic}", name="fs")
                nc.vector.tensor_copy(out=t[:w, :], in_=pfs[:w, :OUT])
                fs.append(t)
            return fs

        fs_u = side_prep(uw1t_sb, ub1_sb, sfu_sb, uw2t_sb, "u")
        fs_v = side_prep(vw1t_sb, vb1_sb, sfv_sb, vw2t_sb, "v")

        # ---------- degree factors ----------
        def fac_from_deg(deg_ap, fac, nm):
            d2 = scr.tile([128, CH], FP, tag="d2", name="d2")
            nc.vector.tensor_scalar(
                out=d2[:, :], in0=deg_ap, scalar1=1.0, scalar2=None, op0=ALU.max
            )
            d3 = res.tile([128, CH], FP, tag=f"d3{nm}", name="d3")
            nc.scalar.sqrt(out=d3[:, :], in_=d2[:, :])
            nc.vector.reciprocal(out=fac[:, :], in_=d3[:, :])
            return d3

        def fac_own(fac, nm):
            own = res.tile([128, 4], FP, tag=f"own{nm}", name="own")
            for j in range(4):
                tmp = scr.tile([128, CH], FP, tag="ot", bufs=2, name="tmp")
                nc.vector.tensor_tensor(
                    out=tmp[:, :], in0=fac[:, :],
                    in1=selb_sb[:, j * CH : (j + 1) * CH], op=ALU.mult,
                )
                nc.vector.tensor_reduce(
                    out=own[:, j : j + 1], in_=tmp[:, :],
                    axis=mybir.AxisListType.X, op=ALU.add,
                )
            return own

        zagg = res.tile([128, NCORES * CH], FP, tag="zagg")
        nc.sync.dma_start(
            out=zagg[:, :],
            in_=dram_zcu_ag[:, :].rearrange("(k p) c -> p k c", k=NCORES),
        )
        # one strided reduce sums the 8 rank blocks (stride-CH gather on the
        # DVE side is cheap; only the DMA landing must stay contiguous)
        dega = res.tile([128, CH], FP, tag="dega")
        nc.vector.tensor_reduce(
            out=dega[:, :].unsqueeze(-1),
            in_=zagg[:, :].rearrange("p (k c) -> p c k", k=NCORES),
            axis=mybir.AxisListType.X, op=ALU.add,
        )
        afac = res.tile([128, CH], FP, tag="faca", name="afac")
        d3a = fac_from_deg(dega[:, :], afac, "a")
        afac_own = None

        # ---------- Hv chain (items out; contraction over all users) ---------
        ps_hv = [ps_chain.tile([128, SO], FP, tag="chain", bufs=4, name="hv") for _ in range(2)]
        bfac = None
        bfac_own = None
        for r in range(R):
            for c in range(CH):
                msk = scr.tile([128, SO], BF, tag="msk", bufs=4, name="msk")
                nc.vector.tensor_scalar(
                    out=msk[:, :], in0=adj_u_sb[c // 8][:, (c % 8) * SP : (c % 8) * SP + SO],
                    scalar1=float(r + 1), scalar2=afac[:, c : c + 1],
                    op0=ALU.is_equal, op1=ALU.mult,
                )
                for mh in range(2):
                    nc.tensor.matmul(
                        ps_hv[mh][:, :],
                        lhsT=wtiles[r][:, c * M + mh * 128 : c * M + (mh + 1) * 128],
                        rhs=msk[:, :],
                        start=(r == 0 and c == 0), stop=(r == R - 1 and c == CH - 1),
                    )
            if r == 1:
                # pass-2 u scale select: needed only ~100us later, emitted here
                # so its 8 DVE ops don't sit between the reciprocal and the
                # first mask on the DVE FIFO at stream start
                afac_own = fac_own(afac, "a")
                d3a_own = fac_own(d3a, "da")
                fsp_u = []
                for ic in range(4):
                    w = min(128, SO - ic * 128)
                    t = res.tile([128, OUT], BF, tag=f"fspu{ic}", name="fsp")
                    nc.vector.tensor_scalar(
                        out=t[:w, :], in0=fs_u[ic][:w, :],
                        scalar1=d3a_own[:w, ic : ic + 1], scalar2=None,
                        op0=ALU.mult,
                    )
                    fsp_u.append(t)
            if r == 2:
                # emit b-side factor chain mid-stream: its AR is long done, so
                # these DVE/ACT ops slot into gaps without stalling the FIFO
                bfac = res.tile([128, CH], FP, tag="facb", name="bfac")
                bback = res.tile([128, CH], FP, tag="bback")
                nc.sync.dma_start(out=bback[:, :], in_=dram_zci_red[:, :])
                d3b = fac_from_deg(bback[:, :], bfac, "b")
                bfac_own = fac_own(bfac, "b")
                # pre-scale the v-side F term by sqrt(deg_own):
                # relu(s*pa + F) == relu(s*(pa + F*sqrt(deg)))
                d3b_own = fac_own(d3b, "db")
                fsp_v = []
                for ic in range(4):
                    w = min(128, SO - ic * 128)
                    t = res.tile([128, OUT], BF, tag=f"fspv{ic}", name="fsp")
                    nc.vector.tensor_scalar(
                        out=t[:w, :], in0=fs_v[ic][:w, :],
                        scalar1=d3b_own[:w, ic : ic + 1], scalar2=None,
                        op0=ALU.mult,
                    )
                    fsp_v.append(t)

        hb_v = []
        for mh in range(2):
            hb = res.tile([128, SO], BF, tag=f"hbv{mh}", name="hbv")
            hb_v.append(hb)
        for ic in range(4):
            w = min(128, SO - ic * 128)
            for mh in range(2):
                nc.scalar.activation(
                    out=hb_v[mh][:, ic * 128 : ic * 128 + w],
                    in_=ps_hv[mh][:, ic * 128 : ic * 128 + w], func=AF.Relu,
                )

        # wi[1..4] DMAs ride the sync queue: their WAR waits (wpool reuse)
        # stall only sync, never the PE stream.
        for r in range(1, R):
            t = wpool.tile([128, CH * M], BF, tag="w", name="wt")
            nc.sync.dma_start(out=t[:, :], in_=wi[r, :, :])
            witiles.append(t)

        # ---------- pass 2 (v emitted between the chains: its latency chain
        # hides under the Hu stream; the adds run on the idle gpsimd engine
        # so the DVE mask FIFO is never blocked) ----------
        def pass2(hb, fac_own_t, fsp, o_dram):
            for ic in range(4):
                w = min(128, SO - ic * 128)
                pa = ps_small.tile([128, SP], FP, tag="sm", name="pa")
                for mh in range(2):
                    nc.tensor.matmul(
                        pa[:w, :OUT], lhsT=hb[mh][:, ic * 128 : ic * 128 + w],
                        rhs=dwt_sb[mh][:, :], start=(mh == 0), stop=False,
                    )
                # identity-lhsT matmul accumulates the pre-scaled F term into
                # the same PSUM, so one fused scale+relu finishes the row
                nc.tensor.matmul(
                    pa[:w, :OUT], lhsT=ident_sb[:w, :w], rhs=fsp[ic][:w, :],
                    start=False, stop=True,
                )
                ro = scr.tile([128, OUT], FP, tag="ro", bufs=3, name="ro")
                nc.scalar.activation(
                    out=ro[:w, :], in_=pa[:w, :OUT], func=AF.Relu,
                    scale=fac_own_t[:w, ic : ic + 1],
                )
                q = nc.sync if ic % 2 == 0 else nc.gpsimd
                q.dma_start(
                    out=o_dram[ic * 128 : ic * 128 + w, :], in_=ro[:w, :]
                )

        pass2(hb_v, bfac_own, fsp_v, v_out)

        # ---------- Hu chain (users out; contraction over all items) ---------
        ps_hu = [ps_chain.tile([128, SO], FP, tag="chain", bufs=4, name="hu") for _ in range(2)]
        for r in range(R):
            for c in range(CH):
                msk = scr.tile([128, SO], BF, tag="msk", bufs=4, name="msk")
                nc.vector.tensor_scalar(
                    out=msk[:, :], in0=adj_i_sb[c // 8][:, (c % 8) * SP : (c % 8) * SP + SO],
                    scalar1=float(r + 1), scalar2=bfac[:, c : c + 1],
                    op0=ALU.is_equal, op1=ALU.mult,
                )
                for mh in range(2):
                    nc.tensor.matmul(
                        ps_hu[mh][:, :],
                        lhsT=witiles[r][:, c * M + mh * 128 : c * M + (mh + 1) * 128],
                        rhs=msk[:, :],
                        start=(r == 0 and c == 0), stop=(r == R - 1 and c == CH - 1),
                    )
        hb_u = []
        for mh in range(2):
            hb = res.tile([128, SO], BF, tag=f"hbu{mh}", name="hbu")
            hb_u.append(hb)
        for ic in range(4):
            w = min(128, SO - ic * 128)
            for mh in range(2):
                nc.scalar.activation(
                    out=hb_u[mh][:, ic * 128 : ic * 128 + w],
                    in_=ps_hu[mh][:, ic * 128 : ic * 128 + w], func=AF.Relu,
                )

        pass2(hb_u, afac_own, fsp_u, u_out)

    nc.compile()
    return nc


_CACHE = {}


def _get_program():
    if "nc" not in _CACHE:
        _CACHE["nc"] = build_program()
    return _CACHE["nc"]


def _pack(x):
    """[NP, S] -> [128, CH*S] with element (p, c*S+j) = x[c*128+p, j]."""
    s = x.shape[1]
    return np.ascontiguousarray(
        x.reshape(CH, 128, s).transpose(1, 0, 2).reshape(128, CH * s)
    )


def _pad_groups(x, axis):
    """Pad per-core groups of SO rows/cols to SP along `axis`."""
    x = np.moveaxis(x, axis, 0)
    n = x.shape[0]
    assert n == NCORES * SO
    shp = (NCORES, SO) + x.shape[1:]
    xg = x.reshape(shp)
    pad = [(0, 0)] * xg.ndim
    pad[1] = (0, SP - SO)
    xp = np.pad(xg, pad)
    out = xp.reshape((NCORES * SP,) + x.shape[1:])
    return np.moveaxis(out, 0, axis)


def make_in_maps(inputs):
    import ml_dtypes

    bf16 = ml_dtypes.bfloat16
    adj = np.asarray(inputs["adj_matrix"], dtype=np.int32)
    u_sf = np.asarray(inputs["u_sideFeat"], dtype=np.float32)
    v_sf = np.asarray(inputs["v_sideFeat"], dtype=np.float32)
    msg_W = np.asarray(inputs["msg_W"], dtype=np.float32)
    dense_W = np.asarray(inputs["dense_W"], dtype=np.float32)

    adjp = _pad_groups(_pad_groups(adj.astype(np.float32), 0), 1)  # [NP, NP]
    adjp = adjp.astype(bf16)

    # shared (identical on every core)
    wu_full = _pad_groups(msg_W[:, :, :NU].transpose(0, 2, 1), 1)  # [R, NP, M]
    wi_full = _pad_groups(msg_W[:, :, NU:].transpose(0, 2, 1), 1)
    wu_pack = np.stack([_pack(wu_full[r].astype(bf16)) for r in range(R)])
    wi_pack = np.stack([_pack(wi_full[r].astype(bf16)) for r in range(R)])
    dwt = np.ascontiguousarray(dense_W.T).astype(bf16)
    uw1t = np.ascontiguousarray(np.asarray(inputs["u_W1"], np.float32).T).astype(bf16)
    uw2t = np.ascontiguousarray(np.asarray(inputs["u_W2"], np.float32).T).astype(bf16)
    vw1t = np.ascontiguousarray(np.asarray(inputs["v_W1"], np.float32).T).astype(bf16)
    vw2t = np.ascontiguousarray(np.asarray(inputs["v_W2"], np.float32).T).astype(bf16)
    ub1 = np.asarray(inputs["u_b1"], np.float32).reshape(SIDE, 1)
    vb1 = np.asarray(inputs["v_b1"], np.float32).reshape(SIDE, 1)

    in_maps = []
    for c in range(NCORES):
        sl = slice(c * SP, (c + 1) * SP)
        selb = np.zeros((128, 4 * CH), np.float32)
        for j in range(4):
            selb[:, j * CH + 4 * c + j] = 1.0
        sfu_p = np.zeros((FDIM, SP), np.float32)
        sfu_p[:, :SO] = u_sf[c * SO : (c + 1) * SO].T
        sfv_p = np.zeros((FDIM, SP), np.float32)
        sfv_p[:, :SO] = v_sf[c * SO : (c + 1) * SO].T
        in_maps.append(
            {
                "adj_u": _pack(np.ascontiguousarray(adjp[:, sl])),
                "adj_i": _pack(np.ascontiguousarray(adjp[sl, :].T)),
                "wu": wu_pack,
                "wi": wi_pack,
                "sfu": sfu_p.astype(bf16),
                "sfv": sfv_p.astype(bf16),
                "dwt": dwt,
                "uw1t": uw1t,
                "ub1": ub1,
                "uw2t": uw2t,
                "vw1t": vw1t,
                "vb1": vb1,
                "vw2t": vw2t,
                "selb": selb,
            }
        )
    return in_maps


def assemble(results):
    U = np.empty((NU, OUT), np.float32)
    V = np.empty((NI, OUT), np.float32)
    for c in range(NCORES):
        U[c * SO : (c + 1) * SO] = results[c]["u_out"][:SO]
        V[c * SO : (c + 1) * SO] = results[c]["v_out"][:SO]
    return (U, V)


def kernel(**inputs):
    from concourse.bass_utils import run_bass_kernel_spmd

    nc = _get_program()
    res = run_bass_kernel_spmd(nc, make_in_maps(inputs), core_ids=list(range(NCORES)))
    return assemble(res.results)


# revision 30
# speedup vs baseline: 1.3518x; 1.3518x over previous
"""Trainium2 Bass kernel for the bipartite GNN message-passing encoder.

Math (see reference.py):
  A_r = (adj == r), r = 1..5
  An_r = diag(a) A_r diag(b),  a_u = rsqrt(max(Nu,1)), b_i = rsqrt(max(Nv,1))
  Hu = relu(sum_r An_r @ W_items_r^T)   [NU, M]
  Hv = relu(sum_r An_r^T @ W_users_r^T) [NI, M]
  U  = relu(Hu @ dense_W^T + relu(u_sideFeat @ u_W1^T + u_b1) @ u_W2^T)
  V  = relu(Hv @ dense_W^T + relu(v_sideFeat @ v_W1^T + v_b1) @ v_W2^T)

Sharding ("collective-free" 8-way): core c owns users [500c, 500c+500) and
items [500c, 500c+500), padded per-core to 512 (padded global index
512c + j, NP = 4096 total).  Each core holds the FULL contraction data for
its rows: adj[:, I_c] u-major (for Hv) and adj[U_c, :]^T i-major (for Hu),
plus the full msg_W both ways (host-packed bf16).  It computes
HvT[m, own-items] / HuT[m, own-users] completely locally, so there are NO
big AllReduces: only a 16KB degree AllGather (measured faster than an
AllReduce as the first collective: 14-17us vs 17-22us; the 8-rank sum is
done on-device with one rearranged-AP DMA and a DVE/gpsimd tree) plus one
hidden 16KB AllReduce for the item side.  Degrees ride fused zero-count passes
split across DVE (is_equal 0) and ACT (Relu(1 - adj)), both with a
free-axis accum_out (deg = 4000 - sum of zero counts over the 8 cores;
pad rows read 0 and drop out of every product).  Masks are built on DVE
with the contraction-side degree factor folded in via the dual-op; the
output-side factor is applied in pass 2 per-partition (own-core factors
extracted from the AllReduce result with a data-driven one-hot select,
keeping the program SPMD-uniform).  The matmul stream is 640 back-to-back
500-col matmuls into 2 PSUM banks per chain (measured ~2.0 cols/ns
sustained under the HAM duty throttle; 250-col/4-bank and fp8-adjacency
variants both measured slower).  pass2 for the v side is emitted between
the chains so its latency hides under the Hu stream; the side-feature term
is pre-scaled by sqrt(deg_own) and injected into the pass-2 PSUM with an
identity-lhsT matmul, so one fused scale+relu ACT op finishes each row
(relu(s*pa + F) == relu(s*(pa + F*sqrt(deg)))).

Host-side prep (allowed layout work only: slice/pad/transpose/dtype):
everything arrives bf16, pre-transposed, chunk-packed ([128, 32*S] with
element (p, c*S+j) = src[c*128+p, j]) so every big DMA is one contiguous
descriptor.
"""

import sys

import numpy as np

if "/opt/trn_rl_repo" not in sys.path:
    sys.path.insert(0, "/opt/trn_rl_repo")

import concourse.bacc as bacc  # noqa: E402
import concourse.mybir as mybir  # noqa: E402
import concourse.tile as tile  # noqa: E402
from concourse.masks import make_identity  # noqa: E402

FP = mybir.dt.float32
BF = mybir.dt.bfloat16

NU = NI = 4000
R = 5
M = 256
OUT = 75
SIDE = 64
FDIM = 128

NCORES = 8
SO = 500        # owned users/items per core
SP = 512        # padded owned span
NP = 4096       # padded global span
CH = NP // 128  # 32 chunks of 128 along the contraction dim

AF = mybir.ActivationFunctionType
ALU = mybir.AluOpType
WORLD = [list(range(NCORES))]


def build_program():
    from contextlib import ExitStack

    nc = bacc.Bacc("TRN2", target_bir_lowering=False, debug=False, num_devices=NCORES)

    # ---- I/O ----
    # adj_u: chunk-packed [128, CH*SP] from adj_pad[:, I_c]   (u on partitions)
    # adj_i: chunk-packed [128, CH*SP] from adj_pad[U_c, :]^T (i on partitions)
    adj_u = nc.dram_tensor("adj_u", [128, CH * SP], BF, kind="ExternalInput")
    adj_i = nc.dram_tensor("adj_i", [128, CH * SP], BF, kind="ExternalInput")
    # wu/wi: chunk-packed [R, 128, CH*M] from msg_W slices, pre-transposed
    wu = nc.dram_tensor("wu", [R, 128, CH * M], BF, kind="ExternalInput")
    wi = nc.dram_tensor("wi", [R, 128, CH * M], BF, kind="ExternalInput")
    sfu = nc.dram_tensor("sfu", [FDIM, SP], BF, kind="ExternalInput")
    sfv = nc.dram_tensor("sfv", [FDIM, SP], BF, kind="ExternalInput")
    dwt = nc.dram_tensor("dwt", [M, OUT], BF, kind="ExternalInput")
    uw1t = nc.dram_tensor("uw1t", [FDIM, SIDE], BF, kind="ExternalInput")
    ub1 = nc.dram_tensor("ub1", [SIDE, 1], FP, kind="ExternalInput")
    uw2t = nc.dram_tensor("uw2t", [SIDE, OUT], BF, kind="ExternalInput")
    vw1t = nc.dram_tensor("vw1t", [FDIM, SIDE], BF, kind="ExternalInput")
    vb1 = nc.dram_tensor("vb1", [SIDE, 1], FP, kind="ExternalInput")
    vw2t = nc.dram_tensor("vw2t", [SIDE, OUT], BF, kind="ExternalInput")
    # selb: [128, 4*CH] one-hot select blocks; block j column (4c+j) is 1.0
    selb = nc.dram_tensor("selb", [128, 4 * CH], FP, kind="ExternalInput")
    u_out = nc.dram_tensor("u_out", [SO, OUT], FP, kind="ExternalOutput")
    v_out = nc.dram_tensor("v_out", [SO, OUT], FP, kind="ExternalOutput")

    with tile.TileContext(nc) as tc, ExitStack() as ctx:
        res = ctx.enter_context(tc.tile_pool(name="res", bufs=1))
        scr = ctx.enter_context(tc.tile_pool(name="scr", bufs=2))
        wpool = ctx.enter_context(tc.tile_pool(name="wpool", bufs=6))
        dram = ctx.enter_context(tc.tile_pool(name="dram", bufs=1, space="DRAM"))
        ps_chain = ctx.enter_context(tc.tile_pool(name="ps_chain", bufs=4, space="PSUM"))
        ps_small = ctx.enter_context(tc.tile_pool(name="ps_small", bufs=4, space="PSUM"))

        # ---------- bulk DMA issue (tensor-engine queue; PE idle pre-stream) ----
        # transfer order = priority: adj (degree gate) > smalls > weights
        adj_u_sb = []
        adj_i_sb = []
        for k in range(4):
            t = res.tile([128, 8 * SP], BF, tag=f"adju{k}", name="adju")
            nc.gpsimd.dma_start(out=t[:, :], in_=adj_u[:, k * 8 * SP : (k + 1) * 8 * SP])
            adj_u_sb.append(t)
        for k in range(4):
            t = res.tile([128, 8 * SP], BF, tag=f"adji{k}", name="adji")
            nc.gpsimd.dma_start(out=t[:, :], in_=adj_i[:, k * 8 * SP : (k + 1) * 8 * SP])
            adj_i_sb.append(t)

        sfu_sb = res.tile([FDIM, SP], BF, tag="sfu")
        nc.gpsimd.dma_start(out=sfu_sb[:, :], in_=sfu[:, :])
        sfv_sb = res.tile([FDIM, SP], BF, tag="sfv")
        nc.gpsimd.dma_start(out=sfv_sb[:, :], in_=sfv[:, :])
        dwt_sb = []
        for mh in range(2):
            t = res.tile([128, OUT], BF, tag=f"dwt{mh}")
            nc.gpsimd.dma_start(out=t[:, :], in_=dwt[mh * 128 : (mh + 1) * 128, :])
            dwt_sb.append(t)
        uw1t_sb = res.tile([FDIM, SIDE], BF, tag="uw1t")
        nc.gpsimd.dma_start(out=uw1t_sb[:, :], in_=uw1t[:, :])
        uw2t_sb = res.tile([SIDE, OUT], BF, tag="uw2t")
        nc.gpsimd.dma_start(out=uw2t_sb[:, :], in_=uw2t[:, :])
        vw1t_sb = res.tile([FDIM, SIDE], BF, tag="vw1t")
        nc.gpsimd.dma_start(out=vw1t_sb[:, :], in_=vw1t[:, :])
        vw2t_sb = res.tile([SIDE, OUT], BF, tag="vw2t")
        nc.gpsimd.dma_start(out=vw2t_sb[:, :], in_=vw2t[:, :])
        ub1_sb = res.tile([SIDE, 1], FP, tag="ub1")
        nc.gpsimd.dma_start(out=ub1_sb[:, :], in_=ub1[:, :])
        vb1_sb = res.tile([SIDE, 1], FP, tag="vb1")
        nc.gpsimd.dma_start(out=vb1_sb[:, :], in_=vb1[:, :])
        selb_sb = res.tile([128, 4 * CH], FP, tag="selb")
        nc.gpsimd.dma_start(out=selb_sb[:, :], in_=selb[:, :])
        ident_sb = res.tile([128, 128], BF, tag="ident")
        make_identity(nc, ident_sb[:, :])

        wtiles = []
        for r in range(R):
            t = wpool.tile([128, CH * M], BF, tag="w", name="wt")
            nc.gpsimd.dma_start(out=t[:, :], in_=wu[r, :, :])
            wtiles.append(t)
        witiles = []
        t = wpool.tile([128, CH * M], BF, tag="w", name="wt")
        nc.gpsimd.dma_start(out=t[:, :], in_=wi[0, :, :])
        witiles.append(t)

        # ---------- degree zero-count pass + tiny world AllReduces ------------
        # Split across DVE (is_equal 0) and ACT (Relu(1-x)); both with a
        # free-axis accum_out giving per-row zero counts over the own span.
        # The LAST-started core's zcu path gates the AllReduce (launch skew),
        # so this latency is on the critical path.
        def zc_pass(adj_sb, zc, start=0, upto=CH):
            for c in range(start, upto):
                sl = adj_sb[c // 8][:, (c % 8) * SP : (c % 8) * SP + SO]
                if c % 8 < 5:
                    tscr = scr.tile([128, SO], FP, tag="tscrv", bufs=3, name="tscr")
                    nc.vector.tensor_scalar(
                        out=tscr[:, :], in0=sl, scalar1=0.0, scalar2=None,
                        op0=ALU.is_equal, op1=ALU.add, accum_out=zc[:, c : c + 1],
                    )
                else:
                    tscr = scr.tile([128, SO], FP, tag="tscrs", bufs=3, name="tscr")
                    nc.scalar.activation(
                        out=tscr[:, :], in_=sl,
                        func=AF.Relu, scale=-1.0, bias=1.0,
                        accum_out=zc[:, c : c + 1],
                    )

        zcu = res.tile([128, CH], FP, tag="zcu")
        zci = res.tile([128, CH], FP, tag="zci")
        zc_pass(adj_u_sb, zcu)
        # contribute (SO - zc): the gathered sum is then the degree directly
        zcun = res.tile([128, CH], FP, tag="zcun")
        nc.vector.tensor_scalar(
            out=zcun[:, :], in0=zcu[:, :], scalar1=-1.0, scalar2=float(SO),
            op0=ALU.mult, op1=ALU.add,
        )
        dram_zcu = dram.tile([128, CH], FP, tag="dram_zcu")
        dram_zcu_ag = dram.tile([NCORES * 128, CH], FP, tag="dram_zcu_ag")
        nc.sync.dma_start(out=dram_zcu[:, :], in_=zcun[:, :])
        nc.gpsimd.collective_compute(
            "AllGather", ALU.bypass, replica_groups=WORLD,
            ins=[dram_zcu.opt()], outs=[dram_zcu_ag.opt()],
        )
        zc_pass(adj_i_sb, zci)
        zcin = res.tile([128, CH], FP, tag="zcin")
        nc.vector.tensor_scalar(
            out=zcin[:, :], in0=zci[:, :], scalar1=-1.0, scalar2=float(SO),
            op0=ALU.mult, op1=ALU.add,
        )
        dram_zci = dram.tile([128, CH], FP, tag="dram_zci")
        dram_zci_red = dram.tile([128, CH], FP, tag="dram_zci_red")
        nc.sync.dma_start(out=dram_zci[:, :], in_=zcin[:, :])
        nc.gpsimd.collective_compute(
            "AllReduce", ALU.add, replica_groups=WORLD,
            ins=[dram_zci.opt()], outs=[dram_zci_red.opt()],
        )

        # ---------- side-feature pass-2 prep (independent of collectives) -----
        def side_prep(w1t_sb, b1_sb, sf_sb, w2t_sb, nm):
            pf = ps_small.tile([128, SP], FP, tag="sm", name="pf")
            nc.tensor.matmul(
                pf[:SIDE, :SO], lhsT=w1t_sb[:, :], rhs=sf_sb[:, :SO],
                start=True, stop=True,
            )
            fT = res.tile([SIDE, SO], BF, tag=f"fT{nm}", name="fT")
            nc.scalar.activation(
                out=fT[:, :], in_=pf[:SIDE, :SO], func=AF.Relu, bias=b1_sb[:, :]
            )
            fs = []
            for ic in range(4):
                w = min(128, SO - ic * 128)
                pfs = ps_small.tile([128, SP], FP, tag="sm", name="pfs")
                nc.tensor.matmul(
                    pfs[:w, :OUT], lhsT=fT[:, ic * 128 : ic * 128 + w],
                    rhs=w2t_sb[:, :], start=True, stop=True,
                )
                t = res.tile([128, OUT], FP, tag=f"fs{nm}{ic}", name="fs")
                nc.vector.tensor_copy(out=t[:w, :], in_=pfs[:w, :OUT])
                fs.append(t)
            return fs

        fs_u = side_prep(uw1t_sb, ub1_sb, sfu_sb, uw2t_sb, "u")
        fs_v = side_prep(vw1t_sb, vb1_sb, sfv_sb, vw2t_sb, "v")

        # ---------- degree factors ----------
        def fac_from_deg(deg_ap, fac, nm):
            d2 = scr.tile([128, CH], FP, tag="d2", name="d2")
            nc.vector.tensor_scalar(
                out=d2[:, :], in0=deg_ap, scalar1=1.0, scalar2=None, op0=ALU.max
            )
            d3 = res.tile([128, CH], FP, tag=f"d3{nm}", name="d3")
            nc.scalar.sqrt(out=d3[:, :], in_=d2[:, :])
            nc.vector.reciprocal(out=fac[:, :], in_=d3[:, :])
            return d3

        def fac_own(fac, nm):
            own = res.tile([128, 4], FP, tag=f"own{nm}", name="own")
            for j in range(4):
                tmp = scr.tile([128, CH], FP, tag="ot", bufs=2, name="tmp")
                nc.vector.tensor_tensor(
                    out=tmp[:, :], in0=fac[:, :],
                    in1=selb_sb[:, j * CH : (j + 1) * CH], op=ALU.mult,
                )
                nc.vector.tensor_reduce(
                    out=own[:, j : j + 1], in_=tmp[:, :],
                    axis=mybir.AxisListType.X, op=ALU.add,
                )
            return own

        zagg = res.tile([128, NCORES * CH], FP, tag="zagg")
        nc.sync.dma_start(
            out=zagg[:, :],
            in_=dram_zcu_ag[:, :].rearrange("(k p) c -> p k c", k=NCORES),
        )
        # one strided reduce sums the 8 rank blocks (stride-CH gather on the
        # DVE side is cheap; only the DMA landing must stay contiguous)
        dega = res.tile([128, CH], FP, tag="dega")
        nc.vector.tensor_reduce(
            out=dega[:, :].unsqueeze(-1),
            in_=zagg[:, :].rearrange("p (k c) -> p c k", k=NCORES),
            axis=mybir.AxisListType.X, op=ALU.add,
        )
        afac = res.tile([128, CH], FP, tag="faca", name="afac")
        d3a = fac_from_deg(dega[:, :], afac, "a")
        afac_own = None

        # ---------- Hv chain (items out; contraction over all users) ---------
        ps_hv = [ps_chain.tile([128, SO], FP, tag="chain", bufs=4, name="hv") for _ in range(2)]
        bfac = None
        bfac_own = None
        for r in range(R):
            for c in range(CH):
                msk = scr.tile([128, SO], BF, tag="msk", bufs=4, name="msk")
                nc.vector.tensor_scalar(
                    out=msk[:, :], in0=adj_u_sb[c // 8][:, (c % 8) * SP : (c % 8) * SP + SO],
                    scalar1=float(r + 1), scalar2=afac[:, c : c + 1],
                    op0=ALU.is_equal, op1=ALU.mult,
                )
                for mh in range(2):
                    nc.tensor.matmul(
                        ps_hv[mh][:, :],
                        lhsT=wtiles[r][:, c * M + mh * 128 : c * M + (mh + 1) * 128],
                        rhs=msk[:, :],
                        start=(r == 0 and c == 0), stop=(r == R - 1 and c == CH - 1),
                    )
            if r == 1:
                # pass-2 u scale select: needed only ~100us later, emitted here
                # so its 8 DVE ops don't sit between the reciprocal and the
                # first mask on the DVE FIFO at stream start
                afac_own = fac_own(afac, "a")
                d3a_own = fac_own(d3a, "da")
                fsp_u = []
                for ic in range(4):
                    w = min(128, SO - ic * 128)
                    t = res.tile([128, OUT], BF, tag=f"fspu{ic}", name="fsp")
                    nc.vector.tensor_scalar(
                        out=t[:w, :], in0=fs_u[ic][:w, :],
                        scalar1=d3a_own[:w, ic : ic + 1], scalar2=None,
                        op0=ALU.mult,
                    )
                    fsp_u.append(t)
            if r == 2:
                # emit b-side factor chain mid-stream: its AR is long done, so
                # these DVE/ACT ops slot into gaps without stalling the FIFO
                bfac = res.tile([128, CH], FP, tag="facb", name="bfac")
                bback = res.tile([128, CH], FP, tag="bback")
                nc.sync.dma_start(out=bback[:, :], in_=dram_zci_red[:, :])
                d3b = fac_from_deg(bback[:, :], bfac, "b")
                bfac_own = fac_own(bfac, "b")
                # pre-scale the v-side F term by sqrt(deg_own):
                # relu(s*pa + F) == relu(s*(pa + F*sqrt(deg)))
                d3b_own = fac_own(d3b, "db")
                fsp_v = []
                for ic in range(4):
                    w = min(128, SO - ic * 128)
                    t = res.tile([128, OUT], BF, tag=f"fspv{ic}", name="fsp")
                    nc.vector.tensor_scalar(
                        out=t[:w, :], in0=fs_v[ic][:w, :],
                        scalar1=d3b_own[:w, ic : ic + 1], scalar2=None,
                        op0=ALU.mult,
                    )
                    fsp_v.append(t)

        def hb_evac(ps, nm):
            hb = []
            for mh in range(2):
                t = res.tile([128, SO], BF, tag=f"hb{nm}{mh}", name="hb")
                hb.append(t)
            for ic in range(4):
                w = min(128, SO - ic * 128)
                for mh in range(2):
                    dst = hb[mh][:, ic * 128 : ic * 128 + w]
                    srcp = ps[mh][:, ic * 128 : ic * 128 + w]
                    if (2 * ic + mh) % 2 == 0:
                        nc.scalar.activation(out=dst, in_=srcp, func=AF.Relu)
                    else:
                        nc.vector.tensor_scalar(
                            out=dst, in0=srcp, scalar1=0.0, scalar2=None,
                            op0=ALU.max,
                        )
            return hb

        hb_v = hb_evac(ps_hv, "v")

        # wi[1..4] DMAs ride the sync queue: their WAR waits (wpool reuse)
        # stall only sync, never the PE stream.
        for r in range(1, R):
            t = wpool.tile([128, CH * M], BF, tag="w", name="wt")
            nc.sync.dma_start(out=t[:, :], in_=wi[r, :, :])
            witiles.append(t)

        # ---------- pass 2 (v emitted between the chains: its latency chain
        # hides under the Hu stream; the adds run on the idle gpsimd engine
        # so the DVE mask FIFO is never blocked) ----------
        def pass2(hb, fac_own_t, fsp, o_dram):
            for ic in range(4):
                w = min(128, SO - ic * 128)
                pa = ps_small.tile([128, SP], FP, tag="sm", name="pa")
                for mh in range(2):
                    nc.tensor.matmul(
                        pa[:w, :OUT], lhsT=hb[mh][:, ic * 128 : ic * 128 + w],
                        rhs=dwt_sb[mh][:, :], start=(mh == 0), stop=False,
                    )
                # identity-lhsT matmul accumulates the pre-scaled F term into
                # the same PSUM, so one fused scale+relu finishes the row
                nc.tensor.matmul(
                    pa[:w, :OUT], lhsT=ident_sb[:w, :w], rhs=fsp[ic][:w, :],
                    start=False, stop=True,
                )
                ro = scr.tile([128, OUT], FP, tag="ro", bufs=3, name="ro")
                if ic % 2 == 0:
                    nc.scalar.activation(
                        out=ro[:w, :], in_=pa[:w, :OUT], func=AF.Relu,
                        scale=fac_own_t[:w, ic : ic + 1],
                    )
                else:
                    nc.vector.tensor_scalar(
                        out=ro[:w, :], in0=pa[:w, :OUT],
                        scalar1=fac_own_t[:w, ic : ic + 1], scalar2=0.0,
                        op0=ALU.mult, op1=ALU.max,
                    )
                q = nc.sync if ic % 2 == 0 else nc.gpsimd
                q.dma_start(
                    out=o_dram[ic * 128 : ic * 128 + w, :], in_=ro[:w, :]
                )

        pass2(hb_v, bfac_own, fsp_v, v_out)

        # ---------- Hu chain (users out; contraction over all items) ---------
        ps_hu = [ps_chain.tile([128, SO], FP, tag="chain", bufs=4, name="hu") for _ in range(2)]
        for r in range(R):
            for c in range(CH):
                msk = scr.tile([128, SO], BF, tag="msk", bufs=4, name="msk")
                nc.vector.tensor_scalar(
                    out=msk[:, :], in0=adj_i_sb[c // 8][:, (c % 8) * SP : (c % 8) * SP + SO],
                    scalar1=float(r + 1), scalar2=bfac[:, c : c + 1],
                    op0=ALU.is_equal, op1=ALU.mult,
                )
                for mh in range(2):
                    nc.tensor.matmul(
                        ps_hu[mh][:, :],
                        lhsT=witiles[r][:, c * M + mh * 128 : c * M + (mh + 1) * 128],
                        rhs=msk[:, :],
                        start=(r == 0 and c == 0), stop=(r == R - 1 and c == CH - 1),
                    )
        hb_u = hb_evac(ps_hu, "u")

        pass2(hb_u, afac_own, fsp_u, u_out)

    nc.compile()
    return nc


_CACHE = {}


def _get_program():
    if "nc" not in _CACHE:
        _CACHE["nc"] = build_program()
    return _CACHE["nc"]


def _pack(x):
    """[NP, S] -> [128, CH*S] with element (p, c*S+j) = x[c*128+p, j]."""
    s = x.shape[1]
    return np.ascontiguousarray(
        x.reshape(CH, 128, s).transpose(1, 0, 2).reshape(128, CH * s)
    )


def _pad_groups(x, axis):
    """Pad per-core groups of SO rows/cols to SP along `axis`."""
    x = np.moveaxis(x, axis, 0)
    n = x.shape[0]
    assert n == NCORES * SO
    shp = (NCORES, SO) + x.shape[1:]
    xg = x.reshape(shp)
    pad = [(0, 0)] * xg.ndim
    pad[1] = (0, SP - SO)
    xp = np.pad(xg, pad)
    out = xp.reshape((NCORES * SP,) + x.shape[1:])
    return np.moveaxis(out, 0, axis)


def make_in_maps(inputs):
    import ml_dtypes

    bf16 = ml_dtypes.bfloat16
    adj = np.asarray(inputs["adj_matrix"], dtype=np.int32)
    u_sf = np.asarray(inputs["u_sideFeat"], dtype=np.float32)
    v_sf = np.asarray(inputs["v_sideFeat"], dtype=np.float32)
    msg_W = np.asarray(inputs["msg_W"], dtype=np.float32)
    dense_W = np.asarray(inputs["dense_W"], dtype=np.float32)

    adjp = _pad_groups(_pad_groups(adj.astype(np.float32), 0), 1)  # [NP, NP]
    adjp = adjp.astype(bf16)

    # shared (identical on every core)
    wu_full = _pad_groups(msg_W[:, :, :NU].transpose(0, 2, 1), 1)  # [R, NP, M]
    wi_full = _pad_groups(msg_W[:, :, NU:].transpose(0, 2, 1), 1)
    wu_pack = np.stack([_pack(wu_full[r].astype(bf16)) for r in range(R)])
    wi_pack = np.stack([_pack(wi_full[r].astype(bf16)) for r in range(R)])
    dwt = np.ascontiguousarray(dense_W.T).astype(bf16)
    uw1t = np.ascontiguousarray(np.asarray(inputs["u_W1"], np.float32).T).astype(bf16)
    uw2t = np.ascontiguousarray(np.asarray(inputs["u_W2"], np.float32).T).astype(bf16)
    vw1t = np.ascontiguousarray(np.asarray(inputs["v_W1"], np.float32).T).astype(bf16)
    vw2t = np.ascontiguousarray(np.asarray(inputs["v_W2"], np.float32).T).astype(bf16)
    ub1 = np.asarray(inputs["u_b1"], np.float32).reshape(SIDE, 1)
    vb1 = np.asarray(inputs["v_b1"], np.float32).reshape(SIDE, 1)

    in_maps = []
    for c in range(NCORES):
        sl = slice(c * SP, (c + 1) * SP)
        selb = np.zeros((128, 4 * CH), np.float32)
        for j in range(4):
            selb[:, j * CH + 4 * c + j] = 1.0
        sfu_p = np.zeros((FDIM, SP), np.float32)
        sfu_p[:, :SO] = u_sf[c * SO : (c + 1) * SO].T
        sfv_p = np.zeros((FDIM, SP), np.float32)
        sfv_p[:, :SO] = v_sf[c * SO : (c + 1) * SO].T
        in_maps.append(
            {
                "adj_u": _pack(np.ascontiguousarray(adjp[:, sl])),
                "adj_i": _pack(np.ascontiguousarray(adjp[sl, :].T)),
                "wu": wu_pack,
                "wi": wi_pack,
                "sfu": sfu_p.astype(bf16),
                "sfv": sfv_p.astype(bf16),
                "dwt": dwt,
                "uw1t": uw1t,
                "ub1": ub1,
                "uw2t": uw2t,
                "vw1t": vw1t,
                "vb1": vb1,
                "vw2t": vw2t,
                "selb": selb,
            }
        )
    return in_maps


def assemble(results):
    U = np.empty((NU, OUT), np.float32)
    V = np.empty((NI, OUT), np.float32)
    for c in range(NCORES):
        U[c * SO : (c + 1) * SO] = results[c]["u_out"][:SO]
        V[c * SO : (c + 1) * SO] = results[c]["v_out"][:SO]
    return (U, V)


def kernel(**inputs):
    from concourse.bass_utils import run_bass_kernel_spmd

    nc = _get_program()
    res = run_bass_kernel_spmd(nc, make_in_maps(inputs), core_ids=list(range(NCORES)))
    return assemble(res.results)
